# revision 2
# baseline (speedup 1.0000x reference)
"""Two-layer GCN (PyG GCNConv) on 8 Trainium2 NeuronCores — v2.

Structure (per core, dst-sharded, 98 blocks of 128 nodes):
  p = dis[n] * (x @ W1)  -> bf16 row table, AllGather -> p_table [NPAD,128]
  S[ch, d] = sum_e p_table[src_e, ch]    (edges grouped by (block-pair G, chunk c);
      dma_gather per (G,c); one-hot scatter matmuls accumulate per dst-block in
      PSUM, block-major across chunks so PSUM holds the full block sum)
  h = relu(S * dis[dst] + b)   (DVE column-scale via broadcast dis + ACT relu-bias)
  ... same for layer 2, then outT = Wfc^T @ h2 + bfc.

Key changes vs v1: dis[dst] applied once per block instead of per edge (one-hots
are pure is_equal); one-hots built in batches of 16 via a single DVE tensor_tensor
with stride-0 broadcast APs; PSUM accumulates whole blocks (no SBUF Z buffer);
idx shipped [16, S/16] and expanded on-device; host prep fully vectorized.
"""

import os
import sys

sys.path.insert(0, "/opt/trn_rl_repo")

from contextlib import ExitStack
from dataclasses import dataclass

import numpy as np
import ml_dtypes

import concourse.bacc as bacc
import concourse.tile as tile
import concourse.mybir as mybir
from concourse.bass import AP
from concourse.bass_utils import run_bass_kernel_spmd
from concourse.library_config import mlp

F32 = mybir.dt.float32
BF16 = mybir.dt.bfloat16
I16 = mybir.dt.int16

_NQUEUES = int(os.environ.get("NQUEUES", "1"))
_SINGLE_PACKET = bool(int(os.environ.get("SINGLE_PACKET", "0")))
_OHB = int(os.environ.get("OHB", "16"))       # one-hots per DVE instruction
_GBUFS = int(os.environ.get("GBUFS", "6"))
_SKIP_GATHER = bool(int(os.environ.get("SKIP_GATHER", "0")))
_SKIP_MM = bool(int(os.environ.get("SKIP_MM", "0")))
_SKIP_OH = bool(int(os.environ.get("SKIP_OH", "0")))


@dataclass(frozen=True)
class Cfg:
    n: int = 100000
    nc: int = 8
    blk: int = 128
    bpc: int = 98          # blocks per core; npad = 100352
    nchunks: int = 4
    bg: int = int(os.environ.get("BG", "2"))   # blocks per gather group

    @property
    def npad(self):
        return self.nc * self.bpc * self.blk

    @property
    def nodes_pc(self):
        return self.bpc * self.blk

    @property
    def chunk_rows(self):
        return self.npad // self.nchunks

    @property
    def ngroups(self):
        return self.bpc // self.bg


CFG = Cfg()


def _prep(cfg: Cfg, edge_index: np.ndarray):
    """Vectorized host-side prep. Returns (meta, idx_w, dstloc, dis)."""
    n, npad, blk = cfg.n, cfg.npad, cfg.blk
    npc, ncr = cfg.nodes_pc, cfg.chunk_rows
    nch, nG, BG, bpc, ncore = cfg.nchunks, cfg.ngroups, cfg.bg, cfg.bpc, cfg.nc

    src = np.asarray(edge_index[0]).astype(np.int64)
    dst = np.asarray(edge_index[1]).astype(np.int64)
    loops = np.arange(n, dtype=np.int64)
    s = np.concatenate([src, loops])
    d = np.concatenate([dst, loops])

    deg = np.bincount(d, minlength=n).astype(np.float64)
    dis = np.zeros(npad, np.float32)
    dis[:n] = (1.0 / np.sqrt(np.maximum(deg, 1.0))).astype(np.float32)

    core = d // npc
    b = (d % npc) // blk
    g = b // BG
    c = s // ncr

    kk = ((core * nG + g) * nch + c) * bpc + b
    order = np.argsort(kk * (1 << 17) + s)   # no stability needed
    s, d = s[order], d[order]
    core, b, g, c = core[order], b[order], g[order], c[order]

    gid = (core * nG + g) * nch + c
    cnt_gc = np.bincount(gid, minlength=ncore * nG * nch).reshape(ncore, nG, nch)
    bid = gid * BG + (b % BG)
    cnt_b = np.bincount(bid, minlength=ncore * nG * nch * BG).reshape(
        ncore, nG, nch, BG
    )

    T = np.maximum(-(-cnt_gc.max(axis=0) // blk), 1)     # [nG, nch]
    S = int(T.sum()) * blk

    Tflat = T.reshape(-1)
    slot_off = np.zeros(nG * nch, np.int64)
    np.cumsum(Tflat[:-1] * blk, out=slot_off[1:])
    slot_off = slot_off.reshape(nG, nch)

    start = np.zeros(ncore * nG * nch + 1, np.int64)
    np.cumsum(cnt_gc.reshape(-1), out=start[1:])
    rank = np.arange(len(s), dtype=np.int64) - start[gid]
    slot = slot_off[g, c] + rank

    idx16 = np.zeros((ncore, S), np.int16)
    idx16[core, slot] = (s - c * ncr).astype(np.int16)

    # union tile spans for the two blocks of each group
    n0 = cnt_b[:, :, :, 0]
    hi0 = -(-n0.max(axis=0) // blk)
    lo1 = n0.min(axis=0) // blk
    hi0 = np.minimum(np.maximum(hi0, 1), T)
    lo1 = np.minimum(lo1, T - 1)
    if BG == 1:
        hi0 = T.copy()

    mm_entries = []        # per group: list of (c, t, bl)
    mm_g, mm_c, mm_t, mm_bl = [], [], [], []
    for gg in range(nG):
        entries = []
        for bl in range(BG):
            seq = []
            for cc in range(nch):
                t0 = 0 if bl == 0 else int(lo1[gg, cc])
                t1 = int(hi0[gg, cc]) if bl == 0 else int(T[gg, cc])
                for tt in range(t0, t1):
                    seq.append((cc, tt, bl))
            assert seq, (gg, bl)
            entries.append(seq)
        mm_entries.append(entries)
        for seq in entries:
            for (cc, tt, bl) in seq:
                mm_g.append(gg); mm_c.append(cc); mm_t.append(tt); mm_bl.append(bl)
    nmm = len(mm_g)
    mm_g = np.array(mm_g); mm_c = np.array(mm_c)
    mm_t = np.array(mm_t); mm_bl = np.array(mm_bl)

    mmcol = np.full((nG, nch, int(T.max()), BG), -1, np.int64)
    mmcol[mm_g, mm_c, mm_t, mm_bl] = np.arange(nmm)

    tile_i = rank // blk
    lane = rank % blk
    col = mmcol[g, c, tile_i, b % BG]
    assert (col >= 0).all(), "edge outside union tile span"

    # scatter in [core, col, lane] order (edges are ~sorted by (col, lane):
    # good locality), then transpose to the device layout [core, lane, col]
    dstloc_cl = np.full((ncore, nmm, blk), 255, np.uint8)
    dstloc_cl[core, col, lane] = (d % blk).astype(np.uint8)
    dstloc = np.ascontiguousarray(dstloc_cl.transpose(0, 2, 1))

    idx_w = np.ascontiguousarray(idx16.reshape(ncore, S // 16, 16).transpose(0, 2, 1))

    meta = {
        "T": T, "S": S, "nmm": nmm, "mm_entries": mm_entries,
        "slot_off": slot_off, "bg": BG,
    }
    return meta, idx_w, dstloc, dis


def _bc3(ap2d, nb, axis_free_first: bool):
    """[p, F] AP -> [p, nb, F] (broadcast middle) or [p, F0, blk]-style views.

    axis_free_first=True: result [p, nb, F] with middle dim stride 0.
    axis_free_first=False: result [p, F, nb]?? (unused)
    """
    a = ap2d.ap
    return AP(ap2d.tensor, ap2d.offset, [list(a[0]), [0, nb], list(a[1])])


def _expand_cols(ap2d, nb, blk):
    """dstloc slice [p, nb] -> [p, nb, blk] with last dim stride 0."""
    a = ap2d.ap
    return AP(ap2d.tensor, ap2d.offset, [list(a[0]), list(a[1]), [0, blk]])


def _build(cfg: Cfg, meta, ohb=None, gbufs=None, nqueues=None, single_packet=None):
    ohb = _OHB if ohb is None else ohb
    gbufs = _GBUFS if gbufs is None else gbufs
    nqueues = _NQUEUES if nqueues is None else nqueues
    single_packet = _SINGLE_PACKET if single_packet is None else single_packet
    T, S, nmm = meta["T"], meta["S"], meta["nmm"]
    mm_entries, slot_off = meta["mm_entries"], meta["slot_off"]
    nG, nch, BG = cfg.ngroups, cfg.nchunks, cfg.bg
    blk, bpc, npc, ncr = cfg.blk, cfg.bpc, cfg.nodes_pc, cfg.chunk_rows
    Tmax = int(T.max())

    nc = bacc.Bacc(
        "TRN2", target_bir_lowering=False, debug=False, num_devices=cfg.nc,
        num_swdge_queues=max(1, nqueues),
    )

    xT_d = nc.dram_tensor("xT", [4, npc], F32, kind="ExternalInput")
    w1_d = nc.dram_tensor("w1", [4, 128], F32, kind="ExternalInput")
    b1_d = nc.dram_tensor("b1", [128, 1], F32, kind="ExternalInput")
    w2_d = nc.dram_tensor("w2", [128, 128], F32, kind="ExternalInput")
    b2_d = nc.dram_tensor("b2", [128, 1], F32, kind="ExternalInput")
    wfc_d = nc.dram_tensor("wfc", [128, 4], F32, kind="ExternalInput")
    bfc_d = nc.dram_tensor("bfc", [4, 1], F32, kind="ExternalInput")
    dis_d = nc.dram_tensor("dis", [128, bpc], F32, kind="ExternalInput")
    disrow_d = nc.dram_tensor("disrow", [1, npc], F32, kind="ExternalInput")
    iota_d = nc.dram_tensor("iota", [128, 128], BF16, kind="ExternalInput")
    idxw_d = nc.dram_tensor("idxw", [16, S // 16], I16, kind="ExternalInput")
    dstloc_d = nc.dram_tensor("dstloc", [128, nmm], mybir.dt.uint8, kind="ExternalInput")
    reps_d = nc.dram_tensor("reps", [1, 1], mybir.dt.int32, kind="ExternalInput")
    outT_d = nc.dram_tensor("outT", [4, npc], F32, kind="ExternalOutput")

    with tile.TileContext(nc) as tc, ExitStack() as ctx:
        dram = ctx.enter_context(tc.tile_pool(name="dram", bufs=1, space="DRAM"))
        const = ctx.enter_context(tc.tile_pool(name="const", bufs=1))
        xblk = ctx.enter_context(tc.tile_pool(name="xblk", bufs=4))
        ppsum = ctx.enter_context(tc.tile_pool(name="ppsum", bufs=2, space="PSUM"))
        pout = ctx.enter_context(tc.tile_pool(name="pout", bufs=4))
        idxp = ctx.enter_context(tc.tile_pool(name="idxp", bufs=4))
        gpool = ctx.enter_context(tc.tile_pool(name="gpool", bufs=gbufs))
        ohp = ctx.enter_context(tc.tile_pool(name="ohp", bufs=4))
        apsum = ctx.enter_context(tc.tile_pool(name="apsum", bufs=4, space="PSUM"))
        ztmp = ctx.enter_context(tc.tile_pool(name="ztmp", bufs=4))
        hpool = ctx.enter_context(tc.tile_pool(name="hpool", bufs=1))
        opsum = ctx.enter_context(tc.tile_pool(name="opsum", bufs=2, space="PSUM"))
        outp = ctx.enter_context(tc.tile_pool(name="outp", bufs=4))

        idxbig = dram.tile([128, S // 16], I16)
        p_bounce = dram.tile([npc, 128], BF16)
        p_table = dram.tile([cfg.npad, 128], BF16)
        g_bounce = dram.tile([npc, 128], BF16)
        g_table = dram.tile([cfg.npad, 128], BF16)

        nc.gpsimd.load_library(mlp)

        iota_t = const.tile([128, 128], BF16)
        nc.sync.dma_start(iota_t[:], iota_d[:, :])
        dis_t = const.tile([128, bpc], F32)
        nc.sync.dma_start(dis_t[:], dis_d[:, :])
        dstloc_u8 = const.tile([128, nmm], mybir.dt.uint8)
        nc.sync.dma_start(dstloc_u8[:], dstloc_d[:, :])
        dstloc_t = const.tile([128, nmm], BF16)
        nc.vector.tensor_copy(dstloc_t[:], dstloc_u8[:])
        b1_t = const.tile([128, 1], F32)
        nc.sync.dma_start(b1_t[:], b1_d[:, :])
        b2_t = const.tile([128, 1], F32)
        nc.sync.dma_start(b2_t[:], b2_d[:, :])
        bfc_t = const.tile([4, 1], F32)
        nc.sync.dma_start(bfc_t[:], bfc_d[:, :])
        w1_t = const.tile([4, 128], F32)
        nc.sync.dma_start(w1_t[:], w1_d[:, :])
        w2f_t = const.tile([128, 128], F32)
        nc.sync.dma_start(w2f_t[:], w2_d[:, :])
        wfcf_t = const.tile([128, 4], F32)
        nc.sync.dma_start(wfcf_t[:], wfc_d[:, :])

        w2b_t = const.tile([128, 128], BF16)
        nc.vector.tensor_copy(w2b_t[:], w2f_t[:])
        wfcb_t = const.tile([128, 4], BF16)
        nc.vector.tensor_copy(wfcb_t[:], wfcf_t[:])

        # disb: [128, npc] f32, every partition = dis row (for column scaling).
        # Built via rank-1 PE matmuls: ones[1,128]^T @ disrow[1,512-slice].
        disb = const.tile([128, npc], BF16)
        disrow_t = const.tile([1, npc], F32)
        nc.sync.dma_start(disrow_t[:], disrow_d[:, :])
        ones_t = const.tile([1, 128], F32)
        nc.vector.memset(ones_t[:], 1.0)
        for o in range(0, npc, 128):
            psb = ppsum.tile([128, 128], F32, tag="ps")
            nc.tensor.matmul(
                psb[:], ones_t[:], disrow_t[:, o : o + 128], start=True, stop=True
            )
            nc.vector.tensor_copy(disb[:, o : o + 128], psb[:])

        # expand wrapped idx to 128 partitions in DRAM
        for k in range(8):
            nc.sync.dma_start(idxbig[16 * k : 16 * (k + 1), :], idxw_d[:, :])

        reps_t = const.tile([1, 1], mybir.dt.int32)
        nc.sync.dma_start(reps_t[:], reps_d[:, :])
        reps_val = nc.values_load(
            reps_t[:], min_val=1, max_val=1 << 20, skip_runtime_bounds_check=True
        )

        def table_build(hsrc, bounce, kind):
            for b in range(bpc):
                sl = slice(b * blk, (b + 1) * blk)
                ps = ppsum.tile([128, 128], F32)
                if kind == "p":
                    xb = xblk.tile([4, blk], F32)
                    nc.sync.dma_start(xb[:], xT_d[:, sl])
                    nc.tensor.matmul(ps[:], xb[:], w1_t[:], start=True, stop=True)
                else:
                    nc.tensor.matmul(
                        ps[:], hsrc[:, sl], w2b_t[:], start=True, stop=True
                    )
                pb = pout.tile([128, 128], BF16)
                nc.scalar.mul(pb[:], ps[:], dis_t[:, b : b + 1])
                nc.sync.dma_start(bounce[sl, :], pb[:])

        _call = [0]

        def agg_layer(table, hT, bias_t):
            # one-hot batch tiles, indexed by mm column // _OHB
            oh_tiles: dict = {}

            def get_oh(colidx):
                if _SKIP_OH:
                    return iota_t[:]
                bi = colidx // ohb
                if bi not in oh_tiles:
                    c0 = bi * ohb
                    nb = min(ohb, nmm - c0)
                    oht = ohp.tile([128, nb, 128], BF16)
                    nc.vector.tensor_tensor(
                        oht[:],
                        _bc3(iota_t[:], nb, True),
                        _expand_cols(dstloc_t[:, c0 : c0 + nb], nb, blk),
                        mybir.AluOpType.is_equal,
                    )
                    oh_tiles[bi] = oht
                return oh_tiles[bi][:, colidx % ohb, :]

            col = 0
            colmap = {}
            for gg in range(nG):
                for seq in mm_entries[gg]:
                    for ent in seq:
                        colmap[(gg,) + tuple(ent)] = col
                        col += 1

            for gg in range(nG):
                gts = {}
                for cc in range(nch):
                    if _SKIP_GATHER:
                        break
                    nt = int(T[gg, cc])
                    o8 = int(slot_off[gg, cc]) // 16
                    it = idxp.tile([128, nt * 8], I16)
                    nc.sync.dma_start(it[:], idxbig[:, o8 : o8 + nt * 8])
                    gt = gpool.tile([128, nt, 128], BF16)
                    tbl = table[cc * ncr : (cc + 1) * ncr, :]
                    nc.gpsimd.dma_gather(
                        gt[:], tbl, it[:], nt * blk, nt * blk, 128,
                        single_packet=single_packet,
                        queue_num=_call[0] % nqueues,
                    )
                    _call[0] += 1
                    gts[cc] = gt
                for bl in range(BG):
                    bglob = gg * BG + bl
                    seq = mm_entries[gg][bl]
                    ps = apsum.tile([128, 128], F32)
                    if _SKIP_MM:
                        nc.vector.memset(ps[:], 0.0)
                    for i, (cc, tt, _) in enumerate(seq):
                        if _SKIP_MM:
                            break
                        oh = get_oh(colmap[(gg, cc, tt, bl)])
                        lhs = iota_t[:] if _SKIP_GATHER else gts[cc][:, tt, :]
                        nc.tensor.matmul(
                            ps[:], lhs, oh,
                            start=(i == 0), stop=(i == len(seq) - 1),
                        )
                    sl = slice(bglob * blk, (bglob + 1) * blk)
                    zt = ztmp.tile([128, 128], F32)
                    nc.vector.tensor_tensor(
                        zt[:], ps[:], disb[:, sl], mybir.AluOpType.mult
                    )
                    nc.scalar.activation(
                        hT[:, sl], zt[:], mybir.ActivationFunctionType.Relu,
                        bias=bias_t[:, 0:1],
                    )

        def allgather(src, dst):
            nc.gpsimd.collective_compute(
                "AllGather",
                mybir.AluOpType.bypass,
                replica_groups=[list(range(cfg.nc))],
                ins=[src.opt()],
                outs=[dst.opt()],
            )

        with tc.For_i(0, reps_val, 1, name="repsA"):
            table_build(None, p_bounce, "p")
        allgather(p_bounce, p_table)

        with tc.For_i(0, reps_val, 1, name="repsB"):
            h1T = hpool.tile([128, npc], BF16, tag="hT")
            agg_layer(p_table, h1T, b1_t)
            table_build(h1T, g_bounce, "g")
        allgather(g_bounce, g_table)

        with tc.For_i(0, reps_val, 1, name="repsC"):
            h2T = hpool.tile([128, npc], BF16, tag="hT")
            agg_layer(g_table, h2T, b2_t)
            for b in range(bpc):
                sl = slice(b * blk, (b + 1) * blk)
                ps4 = opsum.tile([4, 128], F32)
                nc.tensor.matmul(
                    ps4[:], wfcb_t[:], h2T[:, sl], start=True, stop=True
                )
                ot = outp.tile([4, 128], F32)
                nc.scalar.add(ot[:], ps4[:], bfc_t[:, 0:1])
                nc.sync.dma_start(outT_d[:, sl], ot[:])

    nc.compile()
    return nc


_CACHE: dict = {}


def _get_program(cfg: Cfg, meta, **knobs):
    key = (cfg, meta["S"], meta["nmm"], tuple(sorted(knobs.items())),
           tuple(tuple(tuple(e) for e in seq) for g in meta["mm_entries"] for seq in g))
    if key not in _CACHE:
        _CACHE[key] = _build(cfg, meta, **knobs)
    return _CACHE[key]


def _make_in_maps(cfg, x, W1, b1, W2, b2, Wfc, bfc, idx_w, dstloc, dis, reps=1):
    n, npc = cfg.n, cfg.nodes_pc
    xT = np.zeros((4, cfg.npad), np.float32)
    xT[:3, :n] = np.asarray(x, np.float32).T
    w1p = np.zeros((4, 128), np.float32)
    w1p[:3] = np.asarray(W1, np.float32)
    wfcp = np.zeros((128, 4), np.float32)
    wfcp[:, :3] = np.asarray(Wfc, np.float32)
    bfcp = np.zeros((4, 1), np.float32)
    bfcp[:3, 0] = np.asarray(bfc, np.float32)
    iota = (
        np.broadcast_to(np.arange(128, dtype=np.float32), (128, 128))
        .astype(ml_dtypes.bfloat16)
        .copy()
    )
    in_maps = []
    for c in range(cfg.nc):
        nsl = slice(c * npc, (c + 1) * npc)
        in_maps.append(
            {
                "xT": np.ascontiguousarray(xT[:, nsl]),
                "w1": w1p,
                "b1": np.asarray(b1, np.float32).reshape(128, 1),
                "w2": np.asarray(W2, np.float32),
                "b2": np.asarray(b2, np.float32).reshape(128, 1),
                "wfc": wfcp,
                "bfc": bfcp,
                "dis": np.ascontiguousarray(dis[nsl].reshape(cfg.bpc, 128).T),
                "disrow": dis[nsl].reshape(1, npc),
                "iota": np.asarray(iota),
                "idxw": idx_w[c],
                "dstloc": np.asarray(dstloc[c]),
                "reps": np.array([[reps]], np.int32),
            }
        )
    return in_maps


_PREP_CACHE: dict = {}


def _prep_cached(cfg, edge_index):
    ei = np.asarray(edge_index)
    key = (ei.shape, hash(ei.tobytes()))
    if key not in _PREP_CACHE:
        _PREP_CACHE[key] = _prep(cfg, ei)
    return _PREP_CACHE[key]


def kernel(x, edge_index, W1, b1, W2, b2, Wfc, bfc):
    cfg = CFG
    meta, idx_w, dstloc, dis = _prep_cached(cfg, edge_index)
    nc = _get_program(cfg, meta)
    in_maps = _make_in_maps(cfg, x, W1, b1, W2, b2, Wfc, bfc, idx_w, dstloc, dis)
    res = run_bass_kernel_spmd(nc, in_maps, core_ids=list(range(cfg.nc)))
    out = np.concatenate(
        [res.results[c]["outT"].T for c in range(cfg.nc)], axis=0
    )
    return np.ascontiguousarray(out[: cfg.n, :3]).astype(np.float32)


# revision 3
# speedup vs baseline: 1.3712x; 1.3712x over previous
"""Two-layer GCN (PyG GCNConv) on 8 Trainium2 NeuronCores — v2.

Structure (per core, dst-sharded, 98 blocks of 128 nodes):
  p = dis[n] * (x @ W1)  -> bf16 row table, AllGather -> p_table [NPAD,128]
  S[ch, d] = sum_e p_table[src_e, ch]    (edges grouped by (block-pair G, chunk c);
      dma_gather per (G,c); one-hot scatter matmuls accumulate per dst-block in
      PSUM, block-major across chunks so PSUM holds the full block sum)
  h = relu(S * dis[dst] + b)   (DVE column-scale via broadcast dis + ACT relu-bias)
  ... same for layer 2, then outT = Wfc^T @ h2 + bfc.

Key changes vs v1: dis[dst] applied once per block instead of per edge (one-hots
are pure is_equal); one-hots built in batches of 16 via a single DVE tensor_tensor
with stride-0 broadcast APs; PSUM accumulates whole blocks (no SBUF Z buffer);
idx shipped [16, S/16] and expanded on-device; host prep fully vectorized.
"""

import os
import sys

sys.path.insert(0, "/opt/trn_rl_repo")

from contextlib import ExitStack
from dataclasses import dataclass

import numpy as np
import ml_dtypes

import concourse.bacc as bacc
import concourse.tile as tile
import concourse.mybir as mybir
from concourse.bass import AP
from concourse.bass_utils import run_bass_kernel_spmd
from concourse.library_config import mlp

F32 = mybir.dt.float32
BF16 = mybir.dt.bfloat16
I16 = mybir.dt.int16

_NQUEUES = int(os.environ.get("NQUEUES", "1"))
_SINGLE_PACKET = bool(int(os.environ.get("SINGLE_PACKET", "0")))
_OHB = int(os.environ.get("OHB", "8"))       # one-hots per DVE instruction
_GBUFS = int(os.environ.get("GBUFS", "6"))
_SKIP_GATHER = bool(int(os.environ.get("SKIP_GATHER", "0")))
_SKIP_MM = bool(int(os.environ.get("SKIP_MM", "0")))
_SKIP_OH = bool(int(os.environ.get("SKIP_OH", "0")))


@dataclass(frozen=True)
class Cfg:
    n: int = 100000
    nc: int = 8
    blk: int = 128
    bpc: int = 98          # blocks per core; npad = 100352
    nchunks: int = 4
    bg: int = int(os.environ.get("BG", "2"))   # blocks per gather group

    @property
    def npad(self):
        return self.nc * self.bpc * self.blk

    @property
    def nodes_pc(self):
        return self.bpc * self.blk

    @property
    def chunk_rows(self):
        return self.npad // self.nchunks

    @property
    def ngroups(self):
        return self.bpc // self.bg


CFG = Cfg()


def _prep(cfg: Cfg, edge_index: np.ndarray):
    """Vectorized host-side prep. Returns (meta, idx_w, dstloc, dis)."""
    n, npad, blk = cfg.n, cfg.npad, cfg.blk
    npc, ncr = cfg.nodes_pc, cfg.chunk_rows
    nch, nG, BG, bpc, ncore = cfg.nchunks, cfg.ngroups, cfg.bg, cfg.bpc, cfg.nc

    src = np.asarray(edge_index[0]).astype(np.int64)
    dst = np.asarray(edge_index[1]).astype(np.int64)
    loops = np.arange(n, dtype=np.int64)
    s = np.concatenate([src, loops])
    d = np.concatenate([dst, loops])

    deg = np.bincount(d, minlength=n).astype(np.float64)
    dis = np.zeros(npad, np.float32)
    dis[:n] = (1.0 / np.sqrt(np.maximum(deg, 1.0))).astype(np.float32)

    core = d // npc
    b = (d % npc) // blk
    g = b // BG
    c = s // ncr

    kk = ((core * nG + g) * nch + c) * bpc + b
    order = np.argsort(kk * (1 << 17) + s)   # no stability needed
    s, d = s[order], d[order]
    core, b, g, c = core[order], b[order], g[order], c[order]

    gid = (core * nG + g) * nch + c
    cnt_gc = np.bincount(gid, minlength=ncore * nG * nch).reshape(ncore, nG, nch)
    bid = gid * BG + (b % BG)
    cnt_b = np.bincount(bid, minlength=ncore * nG * nch * BG).reshape(
        ncore, nG, nch, BG
    )

    T = np.maximum(-(-cnt_gc.max(axis=0) // blk), 1)     # [nG, nch]
    S = int(T.sum()) * blk

    Tflat = T.reshape(-1)
    slot_off = np.zeros(nG * nch, np.int64)
    np.cumsum(Tflat[:-1] * blk, out=slot_off[1:])
    slot_off = slot_off.reshape(nG, nch)

    start = np.zeros(ncore * nG * nch + 1, np.int64)
    np.cumsum(cnt_gc.reshape(-1), out=start[1:])
    rank = np.arange(len(s), dtype=np.int64) - start[gid]
    slot = slot_off[g, c] + rank

    idx16 = np.zeros((ncore, S), np.int16)
    idx16[core, slot] = (s - c * ncr).astype(np.int16)

    # union tile spans for the two blocks of each group
    n0 = cnt_b[:, :, :, 0]
    hi0 = -(-n0.max(axis=0) // blk)
    lo1 = n0.min(axis=0) // blk
    hi0 = np.minimum(np.maximum(hi0, 1), T)
    lo1 = np.minimum(lo1, T - 1)
    if BG == 1:
        hi0 = T.copy()

    mm_entries = []        # per group: list of (c, t, bl)
    mm_g, mm_c, mm_t, mm_bl = [], [], [], []
    for gg in range(nG):
        entries = []
        for bl in range(BG):
            seq = []
            for cc in range(nch):
                t0 = 0 if bl == 0 else int(lo1[gg, cc])
                t1 = int(hi0[gg, cc]) if bl == 0 else int(T[gg, cc])
                for tt in range(t0, t1):
                    seq.append((cc, tt, bl))
            assert seq, (gg, bl)
            entries.append(seq)
        mm_entries.append(entries)
        for seq in entries:
            for (cc, tt, bl) in seq:
                mm_g.append(gg); mm_c.append(cc); mm_t.append(tt); mm_bl.append(bl)
    nmm = len(mm_g)
    mm_g = np.array(mm_g); mm_c = np.array(mm_c)
    mm_t = np.array(mm_t); mm_bl = np.array(mm_bl)

    mmcol = np.full((nG, nch, int(T.max()), BG), -1, np.int64)
    mmcol[mm_g, mm_c, mm_t, mm_bl] = np.arange(nmm)

    tile_i = rank // blk
    lane = rank % blk
    col = mmcol[g, c, tile_i, b % BG]
    assert (col >= 0).all(), "edge outside union tile span"

    # scatter in [core, col, lane] order (edges are ~sorted by (col, lane):
    # good locality), then transpose to the device layout [core, lane, col]
    dstloc_cl = np.full((ncore, nmm, blk), 255, np.uint8)
    dstloc_cl[core, col, lane] = (d % blk).astype(np.uint8)
    dstloc = np.ascontiguousarray(dstloc_cl.transpose(0, 2, 1))

    idx_w = np.ascontiguousarray(idx16.reshape(ncore, S // 16, 16).transpose(0, 2, 1))

    meta = {
        "T": T, "S": S, "nmm": nmm, "mm_entries": mm_entries,
        "slot_off": slot_off, "bg": BG,
    }
    return meta, idx_w, dstloc, dis


def _bc3(ap2d, nb, axis_free_first: bool):
    """[p, F] AP -> [p, nb, F] (broadcast middle) or [p, F0, blk]-style views.

    axis_free_first=True: result [p, nb, F] with middle dim stride 0.
    axis_free_first=False: result [p, F, nb]?? (unused)
    """
    a = ap2d.ap
    return AP(ap2d.tensor, ap2d.offset, [list(a[0]), [0, nb], list(a[1])])


def _expand_cols(ap2d, nb, blk):
    """dstloc slice [p, nb] -> [p, nb, blk] with last dim stride 0."""
    a = ap2d.ap
    return AP(ap2d.tensor, ap2d.offset, [list(a[0]), list(a[1]), [0, blk]])


def _build(cfg: Cfg, meta, ohb=None, gbufs=None, nqueues=None, single_packet=None):
    ohb = _OHB if ohb is None else ohb
    gbufs = _GBUFS if gbufs is None else gbufs
    nqueues = _NQUEUES if nqueues is None else nqueues
    single_packet = _SINGLE_PACKET if single_packet is None else single_packet
    T, S, nmm = meta["T"], meta["S"], meta["nmm"]
    mm_entries, slot_off = meta["mm_entries"], meta["slot_off"]
    nG, nch, BG = cfg.ngroups, cfg.nchunks, cfg.bg
    blk, bpc, npc, ncr = cfg.blk, cfg.bpc, cfg.nodes_pc, cfg.chunk_rows
    Tmax = int(T.max())

    nc = bacc.Bacc(
        "TRN2", target_bir_lowering=False, debug=False, num_devices=cfg.nc,
        num_swdge_queues=max(1, nqueues),
    )

    xT_d = nc.dram_tensor("xT", [4, npc], F32, kind="ExternalInput")
    w1_d = nc.dram_tensor("w1", [4, 128], F32, kind="ExternalInput")
    b1_d = nc.dram_tensor("b1", [128, 1], F32, kind="ExternalInput")
    w2_d = nc.dram_tensor("w2", [128, 128], F32, kind="ExternalInput")
    b2_d = nc.dram_tensor("b2", [128, 1], F32, kind="ExternalInput")
    wfc_d = nc.dram_tensor("wfc", [128, 4], F32, kind="ExternalInput")
    bfc_d = nc.dram_tensor("bfc", [4, 1], F32, kind="ExternalInput")
    dis_d = nc.dram_tensor("dis", [128, bpc], F32, kind="ExternalInput")
    disrow_d = nc.dram_tensor("disrow", [1, npc], F32, kind="ExternalInput")
    iota_d = nc.dram_tensor("iota", [128, 128], BF16, kind="ExternalInput")
    idxw_d = nc.dram_tensor("idxw", [16, S // 16], I16, kind="ExternalInput")
    dstloc_d = nc.dram_tensor("dstloc", [128, nmm], mybir.dt.uint8, kind="ExternalInput")
    reps_d = nc.dram_tensor("reps", [1, 1], mybir.dt.int32, kind="ExternalInput")
    outT_d = nc.dram_tensor("outT", [4, npc], F32, kind="ExternalOutput")

    with tile.TileContext(nc) as tc, ExitStack() as ctx:
        dram = ctx.enter_context(tc.tile_pool(name="dram", bufs=1, space="DRAM"))
        const = ctx.enter_context(tc.tile_pool(name="const", bufs=1))
        xblk = ctx.enter_context(tc.tile_pool(name="xblk", bufs=4))
        ppsum = ctx.enter_context(tc.tile_pool(name="ppsum", bufs=2, space="PSUM"))
        pout = ctx.enter_context(tc.tile_pool(name="pout", bufs=4))
        idxp = ctx.enter_context(tc.tile_pool(name="idxp", bufs=4))
        gpool = ctx.enter_context(tc.tile_pool(name="gpool", bufs=gbufs))
        ohp = ctx.enter_context(tc.tile_pool(name="ohp", bufs=4))
        apsum = ctx.enter_context(tc.tile_pool(name="apsum", bufs=4, space="PSUM"))
        ztmp = ctx.enter_context(tc.tile_pool(name="ztmp", bufs=4))
        hpool = ctx.enter_context(tc.tile_pool(name="hpool", bufs=1))
        opsum = ctx.enter_context(tc.tile_pool(name="opsum", bufs=2, space="PSUM"))
        outp = ctx.enter_context(tc.tile_pool(name="outp", bufs=4))

        idxbig = dram.tile([128, S // 16], I16)
        p_bounce = dram.tile([npc, 128], BF16)
        p_table = dram.tile([cfg.npad, 128], BF16)
        g_bounce = dram.tile([npc, 128], BF16)
        g_table = dram.tile([cfg.npad, 128], BF16)

        nc.gpsimd.load_library(mlp)

        iota_t = const.tile([128, 128], BF16)
        nc.sync.dma_start(iota_t[:], iota_d[:, :])
        dis_t = const.tile([128, bpc], F32)
        nc.sync.dma_start(dis_t[:], dis_d[:, :])
        dstloc_u8 = const.tile([128, nmm], mybir.dt.uint8)
        nc.sync.dma_start(dstloc_u8[:], dstloc_d[:, :])
        dstloc_t = const.tile([128, nmm], BF16)
        nc.vector.tensor_copy(dstloc_t[:], dstloc_u8[:])
        b1_t = const.tile([128, 1], F32)
        nc.sync.dma_start(b1_t[:], b1_d[:, :])
        b2_t = const.tile([128, 1], F32)
        nc.sync.dma_start(b2_t[:], b2_d[:, :])
        bfc_t = const.tile([4, 1], F32)
        nc.sync.dma_start(bfc_t[:], bfc_d[:, :])
        w1_t = const.tile([4, 128], F32)
        nc.sync.dma_start(w1_t[:], w1_d[:, :])
        w2f_t = const.tile([128, 128], F32)
        nc.sync.dma_start(w2f_t[:], w2_d[:, :])
        wfcf_t = const.tile([128, 4], F32)
        nc.sync.dma_start(wfcf_t[:], wfc_d[:, :])

        w2b_t = const.tile([128, 128], BF16)
        nc.vector.tensor_copy(w2b_t[:], w2f_t[:])
        wfcb_t = const.tile([128, 4], BF16)
        nc.vector.tensor_copy(wfcb_t[:], wfcf_t[:])

        # disb: [128, npc] bf16, every partition = dis row (for column
        # scaling). Rank-1 PE matmuls ones[1,128]^T @ disrow-piece; pieces are
        # staged through a tiny recycled pool ([1,N] SBUF tiles pay N bytes on
        # every partition, so a persistent [1, npc] tile would cost 49KB/part).
        disb = const.tile([128, npc], BF16)
        ones_t = const.tile([1, 128], F32)
        nc.vector.memset(ones_t[:], 1.0)
        for o in range(0, npc, 512):
            w = min(512, npc - o)
            dr = xblk.tile([1, w], F32, tag="dr")
            nc.sync.dma_start(dr[:], disrow_d[:, o : o + w])
            psb = ppsum.tile([128, w], F32, tag="ps")
            nc.tensor.matmul(psb[:], ones_t[:], dr[:], start=True, stop=True)
            nc.vector.tensor_copy(disb[:, o : o + w], psb[:])

        # expand wrapped idx to 128 partitions in DRAM
        for k in range(8):
            nc.sync.dma_start(idxbig[16 * k : 16 * (k + 1), :], idxw_d[:, :])

        reps_t = const.tile([1, 1], mybir.dt.int32)
        nc.sync.dma_start(reps_t[:], reps_d[:, :])
        reps_val = nc.values_load(
            reps_t[:], min_val=1, max_val=1 << 20, skip_runtime_bounds_check=True
        )

        def table_build(hsrc, bounce, kind):
            for b in range(bpc):
                sl = slice(b * blk, (b + 1) * blk)
                ps = ppsum.tile([128, 128], F32)
                if kind == "p":
                    xb = xblk.tile([4, blk], F32)
                    nc.sync.dma_start(xb[:], xT_d[:, sl])
                    nc.tensor.matmul(ps[:], xb[:], w1_t[:], start=True, stop=True)
                else:
                    nc.tensor.matmul(
                        ps[:], hsrc[:, sl], w2b_t[:], start=True, stop=True
                    )
                pb = pout.tile([128, 128], BF16)
                nc.scalar.mul(pb[:], ps[:], dis_t[:, b : b + 1])
                nc.sync.dma_start(bounce[sl, :], pb[:])

        _call = [0]

        def agg_layer(table, hT, bias_t):
            # one-hot batch tiles, indexed by mm column // _OHB
            oh_tiles: dict = {}

            def get_oh(colidx):
                if _SKIP_OH:
                    return iota_t[:]
                bi = colidx // ohb
                if bi not in oh_tiles:
                    c0 = bi * ohb
                    nb = min(ohb, nmm - c0)
                    oht = ohp.tile([128, nb, 128], BF16)
                    nc.vector.tensor_tensor(
                        oht[:],
                        _bc3(iota_t[:], nb, True),
                        _expand_cols(dstloc_t[:, c0 : c0 + nb], nb, blk),
                        mybir.AluOpType.is_equal,
                    )
                    oh_tiles[bi] = oht
                return oh_tiles[bi][:, colidx % ohb, :]

            col = 0
            colmap = {}
            for gg in range(nG):
                for seq in mm_entries[gg]:
                    for ent in seq:
                        colmap[(gg,) + tuple(ent)] = col
                        col += 1

            for gg in range(nG):
                gts = {}
                for cc in range(nch):
                    if _SKIP_GATHER:
                        break
                    nt = int(T[gg, cc])
                    o8 = int(slot_off[gg, cc]) // 16
                    it = idxp.tile([128, nt * 8], I16)
                    nc.sync.dma_start(it[:], idxbig[:, o8 : o8 + nt * 8])
                    gt = gpool.tile([128, nt, 128], BF16)
                    tbl = table[cc * ncr : (cc + 1) * ncr, :]
                    nc.gpsimd.dma_gather(
                        gt[:], tbl, it[:], nt * blk, nt * blk, 128,
                        single_packet=single_packet,
                        queue_num=_call[0] % nqueues,
                    )
                    _call[0] += 1
                    gts[cc] = gt
                for bl in range(BG):
                    bglob = gg * BG + bl
                    seq = mm_entries[gg][bl]
                    ps = apsum.tile([128, 128], F32)
                    if _SKIP_MM:
                        nc.vector.memset(ps[:], 0.0)
                    for i, (cc, tt, _) in enumerate(seq):
                        if _SKIP_MM:
                            break
                        oh = get_oh(colmap[(gg, cc, tt, bl)])
                        lhs = iota_t[:] if _SKIP_GATHER else gts[cc][:, tt, :]
                        nc.tensor.matmul(
                            ps[:], lhs, oh,
                            start=(i == 0), stop=(i == len(seq) - 1),
                        )
                    sl = slice(bglob * blk, (bglob + 1) * blk)
                    zt = ztmp.tile([128, 128], F32)
                    nc.vector.tensor_tensor(
                        zt[:], ps[:], disb[:, sl], mybir.AluOpType.mult
                    )
                    nc.scalar.activation(
                        hT[:, sl], zt[:], mybir.ActivationFunctionType.Relu,
                        bias=bias_t[:, 0:1],
                    )

        def allgather(src, dst):
            nc.gpsimd.collective_compute(
                "AllGather",
                mybir.AluOpType.bypass,
                replica_groups=[list(range(cfg.nc))],
                ins=[src.opt()],
                outs=[dst.opt()],
            )

        with tc.For_i(0, reps_val, 1, name="repsA"):
            table_build(None, p_bounce, "p")
        allgather(p_bounce, p_table)

        with tc.For_i(0, reps_val, 1, name="repsB"):
            h1T = hpool.tile([128, npc], BF16, tag="hT")
            agg_layer(p_table, h1T, b1_t)
            table_build(h1T, g_bounce, "g")
        allgather(g_bounce, g_table)

        with tc.For_i(0, reps_val, 1, name="repsC"):
            h2T = hpool.tile([128, npc], BF16, tag="hT")
            agg_layer(g_table, h2T, b2_t)
            for b in range(bpc):
                sl = slice(b * blk, (b + 1) * blk)
                ps4 = opsum.tile([4, 128], F32)
                nc.tensor.matmul(
                    ps4[:], wfcb_t[:], h2T[:, sl], start=True, stop=True
                )
                ot = outp.tile([4, 128], F32)
                nc.scalar.add(ot[:], ps4[:], bfc_t[:, 0:1])
                nc.sync.dma_start(outT_d[:, sl], ot[:])

    nc.compile()
    return nc


_CACHE: dict = {}


def _get_program(cfg: Cfg, meta, **knobs):
    key = (cfg, meta["S"], meta["nmm"], tuple(sorted(knobs.items())),
           tuple(tuple(tuple(e) for e in seq) for g in meta["mm_entries"] for seq in g))
    if key not in _CACHE:
        _CACHE[key] = _build(cfg, meta, **knobs)
    return _CACHE[key]


def _make_in_maps(cfg, x, W1, b1, W2, b2, Wfc, bfc, idx_w, dstloc, dis, reps=1):
    n, npc = cfg.n, cfg.nodes_pc
    xT = np.zeros((4, cfg.npad), np.float32)
    xT[:3, :n] = np.asarray(x, np.float32).T
    w1p = np.zeros((4, 128), np.float32)
    w1p[:3] = np.asarray(W1, np.float32)
    wfcp = np.zeros((128, 4), np.float32)
    wfcp[:, :3] = np.asarray(Wfc, np.float32)
    bfcp = np.zeros((4, 1), np.float32)
    bfcp[:3, 0] = np.asarray(bfc, np.float32)
    iota = (
        np.broadcast_to(np.arange(128, dtype=np.float32), (128, 128))
        .astype(ml_dtypes.bfloat16)
        .copy()
    )
    in_maps = []
    for c in range(cfg.nc):
        nsl = slice(c * npc, (c + 1) * npc)
        in_maps.append(
            {
                "xT": np.ascontiguousarray(xT[:, nsl]),
                "w1": w1p,
                "b1": np.asarray(b1, np.float32).reshape(128, 1),
                "w2": np.asarray(W2, np.float32),
                "b2": np.asarray(b2, np.float32).reshape(128, 1),
                "wfc": wfcp,
                "bfc": bfcp,
                "dis": np.ascontiguousarray(dis[nsl].reshape(cfg.bpc, 128).T),
                "disrow": dis[nsl].reshape(1, npc),
                "iota": np.asarray(iota),
                "idxw": idx_w[c],
                "dstloc": np.asarray(dstloc[c]),
                "reps": np.array([[reps]], np.int32),
            }
        )
    return in_maps


_PREP_CACHE: dict = {}


def _prep_cached(cfg, edge_index):
    ei = np.asarray(edge_index)
    key = (ei.shape, hash(ei.tobytes()))
    if key not in _PREP_CACHE:
        _PREP_CACHE[key] = _prep(cfg, ei)
    return _PREP_CACHE[key]


def kernel(x, edge_index, W1, b1, W2, b2, Wfc, bfc):
    cfg = CFG
    meta, idx_w, dstloc, dis = _prep_cached(cfg, edge_index)
    nc = _get_program(cfg, meta)
    in_maps = _make_in_maps(cfg, x, W1, b1, W2, b2, Wfc, bfc, idx_w, dstloc, dis)
    res = run_bass_kernel_spmd(nc, in_maps, core_ids=list(range(cfg.nc)))
    out = np.concatenate(
        [res.results[c]["outT"].T for c in range(cfg.nc)], axis=0
    )
    return np.ascontiguousarray(out[: cfg.n, :3]).astype(np.float32)
